# revision 42
# baseline (speedup 1.0000x reference)
"""CrystalGNN message-passing kernel for 8 Trainium2 NeuronCores.

Strategy:
  Host: sort edges by dst node; greedily pack consecutive nodes into
  super-tiles of <=1024 edges and <=128 nodes; assign super-tiles
  contiguously to the 8 cores (one shared SPMD program, no collectives:
  dst-sharding makes per-core aggregates disjoint).  Ship per super-tile
  one bf16 slab [128, 1024] (gathered X_src/X_dst features, feature-major)
  and one fp8 slab [128, 1536] (E features packed 2-up | one-hot(dst)).

  Device (per core), per super-tile (feature-major matmuls, weights
  stationary bf16, edges streaming, fp32 PSUM; every PSUM consumer
  double-buffered so no engine stalls the PE -- keeps PE HAM-warm at
  2.4 GHz; weight-run ordering minimizes LDWEIGHTS row-conflict stalls):
   - attention L1 [192->48] 2-up packed; the E-part MM pair runs
     concurrently in disjoint PE quadrants (tile_position)
   - message L1 [192->128]
   - attention L2 [48->24] 4-up packed (4 concurrent quadrant MMs)
   - attention L3 [24->1] one MM -> [4, 256]; sigmoid on ACT; 2 small PE
     transposes (strided cols) give the per-edge gate column
   - message L2 [128->64] EDGE-major via matmul(lhsT=m1 block, rhs=W2)
     (the LDWEIGHTS does the transpose for free)
   - gate-multiply on DVE (broadcast along free axis); gate stored as a
     65th feature column
   - segment-sum via one-hot matmul: one-hot block is the STATIONARY
     operand (fp8, FWL), gated messages stream -> node-major [128, 65]
     accumulated over 8 edge blocks (col 64 = gate sum for mb2 fixup)

  Host: X_out = X + OUT[col, 0:64] + OUT[col, 64] * mb2
"""

import math
import sys

sys.path.insert(0, "/opt/trn_rl_repo")

import numpy as np

N_CORES = 8
VARIANT = {"pH": 6, "pA": 3, "pS": 3}
DIM = 64
DIM3 = 3 * DIM
SUP_E = 1024
SUP_T = SUP_E // 128
SUP_N = 128
CH = 512
TW = 1024   # bf16 slab cols per super-tile: h1 (X_src | X_dst features)
F8W = 1536  # fp8 slab cols per super-tile: 512 E-packed + 1024 one-hot


def _bf16(x):
    import ml_dtypes
    return np.asarray(x).astype(ml_dtypes.bfloat16)


def _prep(X, E, edge_index):
    """Sort+pack edges into super-tiles; build the per-core slabs."""
    n_nodes = X.shape[0]
    src = np.asarray(edge_index[0]).astype(np.int64)
    dst = np.asarray(edge_index[1]).astype(np.int64)
    n_edges = src.shape[0]

    order = np.argsort(dst, kind="stable")
    dst_s = dst[order]
    src_s = src[order]

    deg = np.bincount(dst, minlength=n_nodes)
    cum = np.zeros(n_nodes + 1, np.int64)
    np.cumsum(deg, out=cum[1:])

    node_lo_list = [0]
    cur_lo = 0
    cur_e = 0
    for n in range(n_nodes):
        d = deg[n]
        if (n - cur_lo) >= SUP_N or cur_e + d > SUP_E:
            node_lo_list.append(n)
            cur_lo = n
            cur_e = 0
        cur_e += d
    node_lo = np.asarray(node_lo_list, np.int64)
    s_total = len(node_lo)
    S = math.ceil(s_total / N_CORES)
    s_pad = S * N_CORES

    node_st = np.searchsorted(node_lo, np.arange(n_nodes), side="right") - 1
    st_of_edge = node_st[dst_s]
    e_start_of_st = cum[node_lo]
    slot = st_of_edge * SUP_E + (np.arange(n_edges) - e_start_of_st[st_of_edge])
    assert slot.max() < s_pad * SUP_E

    import ml_dtypes
    bf = ml_dtypes.bfloat16
    f8 = ml_dtypes.float8_e4m3
    SLAB = np.zeros((s_pad, 128, TW), bf)
    F8S = np.zeros((s_pad, 128, F8W), f8)

    # h1 (features 0..127) straight scatter; E (features 128..191) built
    # into a temp then packed 2-up on partitions as fp8.
    HTb = np.zeros((64, s_pad * SUP_E), f8)
    st_idx = slot // SUP_E
    col_in = slot % SUP_E
    step = 1 << 18
    for i in range(0, n_edges, step):
        sl = slice(i, i + step)
        SLAB[st_idx[sl], 0:64, col_in[sl]] = _bf16(X[src_s[sl]])
        SLAB[st_idx[sl], 64:128, col_in[sl]] = _bf16(X[dst_s[sl]])
        HTb[:, slot[sl]] = E[order[sl]].T.astype(f8)
    hb = HTb.reshape(64, s_pad, 2, CH)
    F8S[:, 0:64, 0:CH] = hb[:, :, 0, :].transpose(1, 0, 2)
    F8S[:, 64:128, 0:CH] = hb[:, :, 1, :].transpose(1, 0, 2)
    del HTb, hb

    # one-hot(dst_local) as fp8 (0/1 exact): col 512 + block*128 + dst_local
    dst_local = (dst_s - node_lo[st_of_edge]).astype(np.int64)
    F8S[slot // SUP_E, slot % 128,
        CH + ((slot % SUP_E) // 128) * 128 + dst_local] = f8(1.0)

    node_col = node_st * 128 + (np.arange(n_nodes) - node_lo[node_st])
    return SLAB, F8S, S, node_col


def _emit(tc, t, S, reps=1):
    import concourse.tile as tile  # noqa: F401
    from concourse import mybir
    from concourse.masks import make_identity
    from contextlib import ExitStack

    nc = tc.nc
    f32 = mybir.dt.float32
    bf16 = mybir.dt.bfloat16
    AF = mybir.ActivationFunctionType
    OP = mybir.AluOpType

    with ExitStack() as ctx:
        cpool = ctx.enter_context(tc.tile_pool(name="const", bufs=1))
        pH = ctx.enter_context(tc.tile_pool(name="hslab", bufs=VARIANT.get("pH", 6)))
        pA = ctx.enter_context(tc.tile_pool(name="acts", bufs=VARIANT.get("pA", 3)))
        pS = ctx.enter_context(tc.tile_pool(name="small", bufs=VARIANT.get("pS", 3)))
        pp1 = ctx.enter_context(tc.tile_pool(name="ps1p", bufs=2, space="PSUM"))
        ppm = ctx.enter_context(tc.tile_pool(name="psmp", bufs=2, space="PSUM"))
        pp2 = ctx.enter_context(tc.tile_pool(name="ps2p", bufs=2, space="PSUM"))
        ppM = ctx.enter_context(tc.tile_pool(name="psMp", bufs=1, space="PSUM"))
        ppA = ctx.enter_context(tc.tile_pool(name="psap", bufs=1, space="PSUM"))

        id4 = cpool.tile([4, 4], f32)
        make_identity(nc, id4[:])

        def cload(name, p, w, dt):
            tl = cpool.tile([p, w], dt, tag=name)
            nc.sync.dma_start(tl[:], t[name][:, :])
            return tl

        w1a = cload("AW1A", 128, 48, bf16)
        w1b = cload("AW1B", 128, 48, bf16)   # rows 0-63 AND 64-127 = aw1[128:]
        w2p = cload("AW2", 112, 24, bf16)
        w3p = cload("AW3", 120, 4, bf16)
        v1a = cload("MW1A", 128, 128, bf16)
        v1b = cload("MW1B", 128, 128, bf16)  # rows 0-63 AND 64-127 = mw1[128:]
        v2 = cload("MW2", 128, 64, bf16)
        b1 = cload("AB1", 112, 1, f32)
        b2 = cload("AB2", 120, 1, f32)
        b3 = cload("AB3", 4, 1, f32)
        c1 = cload("MB1", 128, 1, f32)

        f8 = mybir.dt.float8e4
        SLAB = t["SLAB"]
        F8S = t["F8S"]
        OUT = t["OUT"]

        for s_ in range(S * reps):
            s = s_ % S
            hh = pH.tile([128, TW], bf16, tag="hh")
            nc.sync.dma_start(hh[:], SLAB[s, :, :])
            ff = pH.tile([128, F8W], f8, tag="ff")
            nc.sync.dma_start(ff[:], F8S[s, :, :])

            # --- L1 matmuls, weight-run ordered to minimize LDW stalls ---
            ps1 = pp1.tile([112, CH], f32, tag="ps1")
            psmA = ppm.tile([128, CH], f32, tag="psm")
            psmB = ppm.tile([128, CH], f32, tag="psm")
            nc.tensor.matmul(ps1[0:48, :], w1a[:], hh[:, 0:CH], start=True, stop=False,
                             skip_group_check=True)
            nc.tensor.matmul(ps1[64:112, :], w1a[:], hh[:, CH:SUP_E], start=True, stop=False,
                             skip_group_check=True)
            # E pair: disjoint quadrants -> concurrent
            nc.tensor.matmul(ps1[0:48, :], w1b[0:64, :], ff[0:64, 0:CH],
                             start=False, stop=True, tile_position=(0, 0),
                             skip_group_check=True)
            nc.tensor.matmul(ps1[64:112, :], w1b[64:128, :], ff[64:128, 0:CH],
                             start=False, stop=True, tile_position=(64, 64),
                             skip_group_check=True)
            nc.tensor.matmul(psmA[:], v1a[:], hh[:, 0:CH], start=True, stop=False,
                             skip_group_check=True)
            nc.tensor.matmul(psmB[:], v1a[:], hh[:, CH:SUP_E], start=True, stop=False,
                             skip_group_check=True)
            nc.tensor.matmul(psmA[:], v1b[0:64, :], ff[0:64, 0:CH],
                             start=False, stop=True, tile_position=(0, 0),
                             skip_group_check=True)
            nc.tensor.matmul(psmB[:], v1b[64:128, :], ff[64:128, 0:CH],
                             start=False, stop=True, tile_position=(64, 0),
                             skip_group_check=True)

            a1 = pA.tile([112, CH], bf16, tag="a1")
            nc.scalar.activation(a1[:], ps1[:], AF.Relu, bias=b1[:, 0:1])

            m1 = pA.tile([128, SUP_E], bf16, tag="m1")
            nc.scalar.activation(m1[:, 0:CH], psmA[:], AF.Relu, bias=c1[:, 0:1])
            nc.vector.tensor_scalar(
                out=m1[:, CH:SUP_E], in0=psmB[:], scalar1=c1[:, 0:1], scalar2=0.0,
                op0=OP.add, op1=OP.max,
            )

            # --- attention L2 [48->24], 4 concurrent quadrant MMs ---
            ps2 = pp2.tile([128, 264], f32, tag="ps2")
            nc.tensor.matmul(ps2[0:24, 0:256], w2p[0:48, :], a1[0:48, 0:256],
                             start=True, stop=True, tile_position=(0, 0))
            nc.tensor.matmul(ps2[32:56, 0:256], w2p[0:48, :], a1[0:48, 256:512],
                             start=True, stop=True, tile_position=(0, 32))
            nc.tensor.matmul(ps2[64:88, 0:256], w2p[64:112, :], a1[64:112, 0:256],
                             start=True, stop=True, tile_position=(64, 64))
            nc.tensor.matmul(ps2[96:120, 0:256], w2p[64:112, :], a1[64:112, 256:512],
                             start=True, stop=True, tile_position=(64, 96))

            a2 = pS.tile([120, 256], bf16, tag="a2")
            nc.vector.tensor_scalar(
                out=a2[:], in0=ps2[0:120, 0:256], scalar1=b2[:, 0:1], scalar2=0.0,
                op0=OP.add, op1=OP.max,
            )

            # --- attention L3 [24->1]: overlay rows 0..3 of ps2 cols 0..255 ---
            nc.tensor.matmul(ps2[0:4, 0:256], w3p[:], a2[:, :], start=True, stop=True,
                             skip_group_check=True)

            sg = pS.tile([4, 256], f32, tag="sg")
            nc.scalar.activation(sg[:], ps2[0:4, 0:256], AF.Sigmoid, bias=b3[:, 0:1])

            # gate row -> edge-major columns, strided so col 256+b <-> block b
            nc.tensor.transpose(ps2[0:128, 256:264:2], sg[0:4, 0:128], id4[:])
            nc.tensor.transpose(ps2[0:128, 257:264:2], sg[0:4, 128:256], id4[:])

            # --- message L2 [128->64] edge-major: lhsT = m1 block ---
            psM = ppM.tile([128, SUP_T, 64], f32, tag="psM")
            for b in range(SUP_T):
                nc.tensor.matmul(
                    psM[:, b, :], m1[:, b * 128 : (b + 1) * 128], v2[:],
                    start=True, stop=True,
                )

            # --- gate: medge[:, b, 0:64] = psM * sig_b ; [:, b, 64] = sig_b ---
            medge = pS.tile([128, SUP_T, 65], bf16, tag="medge")
            nc.vector.tensor_copy(medge[:, :, 64], ps2[:, 256:264])
            nc.vector.tensor_tensor(
                out=medge[:, :, 0:64],
                in0=psM[:],
                in1=medge[:, :, 64].unsqueeze(2).to_broadcast([128, SUP_T, 64]),
                op=OP.mult,
            )

            # --- segment-sum: one-hot stationary (fp8, FWL), messages stream ---
            aggp = ppA.tile([128, 65], f32, tag="aggp")
            for b in range(SUP_T):
                nc.tensor.matmul(
                    aggp[:],
                    ff[:, CH + b * 128 : CH + (b + 1) * 128],
                    medge[:, b, :],
                    start=(b == 0),
                    stop=(b == SUP_T - 1),
                )
            aggs = pS.tile([128, 65], f32, tag="aggs")
            nc.vector.tensor_copy(aggs[:], aggp[:])
            nc.sync.dma_start(OUT[s * 128 : (s + 1) * 128, :], aggs[:])


def _build(S, reps=1):
    import concourse.tile as tile
    from concourse import bacc, mybir

    f32 = mybir.dt.float32
    bf16 = mybir.dt.bfloat16
    nc = bacc.Bacc(
        "TRN2", target_bir_lowering=False, debug=False,
        enable_asserts=False, num_devices=N_CORES,
    )
    t = {}
    def din(name, shape, dt):
        t[name] = nc.dram_tensor(name, list(shape), dt, kind="ExternalInput").ap()

    din("SLAB", (S, 128, TW), bf16)
    din("F8S", (S, 128, F8W), mybir.dt.float8e4)
    din("AW1A", (128, 48), bf16); din("AW1B", (128, 48), bf16)
    din("AW2", (112, 24), bf16); din("AW3", (120, 4), bf16)
    din("MW1A", (128, 128), bf16); din("MW1B", (128, 128), bf16); din("MW2", (128, 64), bf16)
    din("AB1", (112, 1), f32); din("AB2", (120, 1), f32); din("AB3", (4, 1), f32)
    din("MB1", (128, 1), f32)
    t["OUT"] = nc.dram_tensor(
        "OUT", [S * 128, 65], f32, kind="ExternalOutput"
    ).ap()

    with tile.TileContext(nc) as tc:
        _emit(tc, t, S, reps)
    nc.compile()
    return nc


def _make_shared(aw1, ab1, aw2, ab2, aw3, ab3, mw1, mb1, mw2, mb2):
    def pack(v, rows, offs, dt=np.float32):
        v = np.asarray(v, np.float32)
        v = v.reshape(v.shape[0], -1) if v.ndim > 1 else v.reshape(-1, 1)
        out = np.zeros((rows, v.shape[1]), np.float32)
        for o in offs:
            out[o : o + v.shape[0], :] = v
        return out if dt is np.float32 else _bf16(out)
    import ml_dtypes
    bf = ml_dtypes.bfloat16
    aw1 = np.asarray(aw1, np.float32)
    mw1 = np.asarray(mw1, np.float32)
    aw3 = np.asarray(aw3, np.float32).reshape(-1)
    w3p = np.zeros((120, 4), np.float32)
    for k in range(4):
        w3p[32 * k : 32 * k + 24, k] = aw3
    return {
        "AW1A": _bf16(aw1[:128]),
        "AW1B": pack(aw1[128:], 128, (0, 64), bf),
        "AW2": pack(aw2, 112, (0, 64), bf),
        "AW3": _bf16(w3p),
        "MW1A": _bf16(mw1[:128]),
        "MW1B": pack(mw1[128:], 128, (0, 64), bf),
        "MW2": _bf16(np.asarray(mw2, np.float32)),
        "AB1": pack(ab1, 112, (0, 64)),
        "AB2": pack(ab2, 120, (0, 32, 64, 96)),
        "AB3": np.full((4, 1), np.float32(np.asarray(ab3).reshape(-1)[0])),
        "MB1": np.asarray(mb1, np.float32).reshape(128, 1),
    }


def kernel(X, E, emb_nodes, emb_edges, edge_index,
           aw1, ab1, aw2, ab2, aw3, ab3, mw1, mb1, mw2, mb2):
    from concourse.bass_utils import run_bass_kernel_spmd

    X = np.ascontiguousarray(np.asarray(X, np.float32))
    E = np.ascontiguousarray(np.asarray(E, np.float32))
    aw1 = np.asarray(aw1, np.float32); aw2 = np.asarray(aw2, np.float32)
    aw3 = np.asarray(aw3, np.float32); mw1 = np.asarray(mw1, np.float32)
    mw2 = np.asarray(mw2, np.float32)

    SLAB, F8S, S, node_col = _prep(X, E, edge_index)

    nc = _build(S)

    shared = _make_shared(aw1, ab1, aw2, ab2, aw3, ab3, mw1, mb1, mw2, mb2)
    in_maps = []
    for c in range(N_CORES):
        m = dict(shared)
        m["SLAB"] = SLAB[c * S : (c + 1) * S]
        m["F8S"] = F8S[c * S : (c + 1) * S]
        in_maps.append(m)

    res = run_bass_kernel_spmd(nc, in_maps, core_ids=list(range(N_CORES)))

    OUT_all = np.concatenate([res.results[c]["OUT"] for c in range(N_CORES)], axis=0)
    mb2f = np.asarray(mb2, np.float32).reshape(-1)
    X_out = (X + OUT_all[node_col, 0:64]
             + OUT_all[node_col, 64][:, None] * mb2f[None, :])
    return X_out.astype(np.float32)
